# revision 49
# baseline (speedup 1.0000x reference)
"""AttnReadout (segment softmax attention readout) Trainium2 kernel.

out[g] = sum_i softmax_within_graph(tanh(x @ W.T + b) @ query)[i] * x[i]

Strategy (8 NeuronCores, data-parallel over nodes):
  - shard x / graph_ptr along N (16384 nodes per core)
  - per core, on device:
      H^T        = (W.T)^T-chunks @ x^T-chunks        (bf16 matmuls, PSUM f32)
      G^T        = tanh(H^T + b)                      (ScalarE, fused bias, f32)
      score      = q^T @ G^T -> pscore4[4, nodes]     (fp32r PE matvec, one
                                                       16-mm chain per sblock)
      score_col  = 4 batched PE transposes [4,128]->[128,4] per superblock
      e          = exp(score - 30)                    (ScalarE)
      Ew[i,gw]   = (iota[gw] == seg_i) * e_i          (DVE, narrow graph window)
      den[gw]   += ones^T @ Ew                        (PE, rides oacc3's chain)
      outT[d,gw]+= x_chunk_d^T @ Ew                   (PE, flipped: x as weights)
  - host: accumulate per-graph partial sums across cores, divide.

Precision: x/W enter the gate matmul as bf16 (measured end-to-end rel err
contribution ~8e-3 vs the 2e-2 budget); the score path (G^T, q) stays
f32/fp32r; Ew and x enter the weighted sum as bf16 (~2e-3).  The exp
shift is a global constant so per-core partials are directly addable.

The weighted sum is FLIPPED relative to the obvious formulation: graph
ids are sorted, graphs average 256+-16 nodes, so a 128-node chunk spans
<=2 graphs.  Using the x chunk as the PE weights and a 16-column Ew
window as the moving operand cuts the streamed columns from 512 to 64
per chunk.  Window placement (graph ~ chunk/2) is validated host-side
against the actual graph_ptr; a dense (win=128) module is compiled as a
fallback if the data ever violates the window.

PSUM bank discipline (8 banks): a regular matmul with start=True zeroes
its ENTIRE bank and opens it; stop=True closes it (later start=False
writes are dropped); transpose writes bypass this state machine.  Hence
one open accumulation chain per bank and nothing else writing it:
  banks 1-2: phpool ring (gate chains)
  bank  3:   pscore4 chains + warm mms (temporally serialized via the
             pspool ring WARs)
  banks 4-7: oacc0-3 (one flipped-wsum chain each; den rides INSIDE
             oacc3's open chain at columns 128-255, start=False always,
             written after the open and before the close)
  bank  8:   accb (transpose writes + exp reads only)
"""

import os

import numpy as np

P = 128          # partitions
D = 512          # feature dim
G = 512          # num graphs
N_CORES = 8
GRP = 512        # nodes per matmul group (moving free dim)
SUP_G = 4        # groups per superblock (DMA granularity)
KC = D // P      # 4 contraction chunks
MC = D // P      # 4 output-dim chunks
SHIFT = 30.0     # exp(score - SHIFT)
WIN = 16         # Ew graph-window columns (normal mode)
WSLACK = 6       # window starts at chunk//2 - WSLACK

_CACHE = {}
LAST_RESULT = None  # BassKernelResults of the most recent kernel() call


def _ws(ci, win):
    """Window start (relative graph id) for global chunk ci."""
    if win >= P:
        return 0
    return min(max(ci // 2 - WSLACK, 0), P - win)


def _const_layout(nch):
    oqv = 0
    obv = oqv + MC
    osg = obv + MC
    oio = osg + nch
    oeye = oio + P
    cw = oeye + SUP_G
    return oqv, obv, osg, oio, oeye, cw


def build_module(shard, win):
    """Build the Bass/Tile module for one core processing `shard` nodes."""
    import concourse.bacc as bacc
    import concourse.bass as bass  # noqa: F401
    import concourse.mybir as mybir
    import concourse.tile as tile

    f32 = mybir.dt.float32
    f32r = mybir.dt.float32r
    bf16 = mybir.dt.bfloat16
    Tanh = mybir.ActivationFunctionType.Tanh
    Exp = mybir.ActivationFunctionType.Exp
    is_equal = mybir.AluOpType.is_equal
    mult = mybir.AluOpType.mult

    SUP = SUP_G * GRP            # nodes per superblock (2048)
    assert shard % SUP == 0
    NS = shard // SUP            # superblocks
    CPS = SUP // P               # 128-node chunks per superblock (16)
    NCH = shard // P             # total chunks
    JP = GRP // P                # chunks per group (4)
    OQV, OBV, OSG, OIO, OEYE, CW = _const_layout(NCH)
    assert NCH + 4 <= 512

    nc = bacc.Bacc("TRN2", target_bir_lowering=False, debug=False, enable_partition_id=False)

    xt = nc.dram_tensor("xt", [P, NS * KC * SUP], bf16, kind="ExternalInput").ap()
    xn = nc.dram_tensor("xn", [P, NS * CPS * D], bf16, kind="ExternalInput").ap()
    wtb = nc.dram_tensor("wtb", [P, KC * D], bf16, kind="ExternalInput").ap()
    cst = nc.dram_tensor("cst", [P, CW], f32r, kind="ExternalInput").ap()
    ovt = nc.dram_tensor("ovt", [P, MC * P], f32, kind="ExternalOutput").ap()
    od = nc.dram_tensor("od", [1, P], f32, kind="ExternalOutput").ap()

    with tile.TileContext(nc) as tc:
        with (
            tc.tile_pool(name="cpool", bufs=1) as cpool,
            tc.tile_pool(name="xtpool", bufs=2) as xtpool,
            tc.tile_pool(name="xnpool", bufs=3) as xnpool,
            tc.tile_pool(name="gtpool", bufs=2) as gtpool,
            tc.tile_pool(name="spool", bufs=2) as spool,
            tc.tile_pool(name="epool", bufs=2) as epool,
            tc.tile_pool(name="ewpool", bufs=12) as ewpool,
            tc.tile_pool(name="opool", bufs=1) as opool,
            tc.tile_pool(name="phpool", bufs=2, space="PSUM") as phpool,
            tc.tile_pool(name="pspool", bufs=1, space="PSUM") as pspool,
            tc.tile_pool(name="paccpool", bufs=1, space="PSUM") as paccpool,
        ):
            # ---- startup: interleave the first superblock's xt pieces
            # with the constants so the first gate matmul starts ASAP ----
            xg0 = xtpool.tile([P, KC * SUP], bf16, name="xtg")
            xgv0 = xg0.rearrange("p (k n) -> p k n", k=KC)
            xt0v = xt[:, 0:KC * SUP].rearrange("p (k n) -> p k n", k=KC)
            nc.sync.dma_start(out=xgv0[:, 0, 0:GRP], in_=xt0v[:, 0, 0:GRP])

            cst_sb = cpool.tile([P, CW], f32r, name="cst_sb")
            nc.sync.dma_start(out=cst_sb, in_=cst)
            wtb_sb = cpool.tile([P, KC * D], bf16, name="wtb_sb")
            wtv = wtb_sb.rearrange("p (k m) -> p k m", k=KC)
            wtbv = wtb.rearrange("p (k m) -> p k m", k=KC)
            nc.sync.dma_start(out=wtv[:, :, 0:P], in_=wtbv[:, :, 0:P])
            for k in range(1, KC):
                nc.sync.dma_start(out=xgv0[:, k, 0:GRP], in_=xt0v[:, k, 0:GRP])
            for m in range(1, MC):
                nc.sync.dma_start(
                    out=wtv[:, :, m * P:(m + 1) * P],
                    in_=wtbv[:, :, m * P:(m + 1) * P],
                )
            nc.sync.dma_start(out=xgv0[:, :, GRP:SUP], in_=xt0v[:, :, GRP:SUP])

            qv_v = cst_sb[:, OQV:OQV + MC]
            bv_v = cst_sb[:, OBV:OBV + MC].bitcast(f32)
            segc_v = cst_sb[:, OSG:OSG + NCH].bitcast(f32)
            iota_v = cst_sb[:, OIO:OIO + P].bitcast(f32)

            id1_sb = cpool.tile([1, 1], f32, name="id1_sb")
            nc.vector.memset(id1_sb, 1.0)
            shift_sb = cpool.tile([P, 1], f32, name="shift_sb")
            nc.vector.memset(shift_sb, -SHIFT)
            ones_bf = cpool.tile([P, 1], bf16, name="ones_bf")
            nc.vector.memset(ones_bf, 1.0)
            warm_sb = cpool.tile([1, 2], f32, name="warm_sb")

            oaccs = [
                paccpool.tile([P, 512], f32, name=f"oacc{m}", space="PSUM")
                for m in range(MC)
            ]
            accb = paccpool.tile([P, NCH + 4], f32, name="accb", space="PSUM")
            den_acc = oaccs[MC - 1][0:1, P:2 * P]

            # ---- engine warm-ups: observe each constant DMA once ----
            wt1 = pspool.tile([1, GRP], f32, name="pscore", space="PSUM")
            nc.tensor.matmul(
                out=wt1[0:1, 0:2],
                lhsT=cst_sb[0:1, OQV:OQV + 1],
                rhs=cst_sb[0:1, 0:2],
                start=True,
                stop=True,
            )
            wt2 = pspool.tile([1, GRP], f32, name="pscore", space="PSUM")
            nc.tensor.matmul(
                out=wt2[0:1, 0:2],
                lhsT=wtb_sb[0:1, 0:1],
                rhs=wtb_sb[0:1, 0:2],
                start=True,
                stop=True,
            )
            nc.vector.tensor_copy(out=warm_sb[0:1, 0:1], in_=segc_v[0:1, 0:1])
            nc.scalar.copy(out=warm_sb[0:1, 1:2], in_=bv_v[0:1, 0:1])

            def load_xt_sb(s):
                xg = xtpool.tile([P, KC * SUP], bf16, name="xtg")
                nc.sync.dma_start(
                    out=xg, in_=xt[:, s * KC * SUP:(s + 1) * KC * SUP]
                )
                return xg.rearrange("p (k n) -> p k n", k=KC)

            def emit_gate(xgv, gi, filler=None):
                """Gate matmuls for one group; after each m-chunk, `filler`
                may emit one chunk's worth of tiny flipped-wsum matmuls so
                their weight loads hide under the next m-chunk's 213ns
                matmuls instead of stalling the matmul pipe."""
                gt = gtpool.tile([P, MC * GRP], f32r, name="gt")
                last_mm = None
                for m in range(MC):
                    ph = phpool.tile([P, GRP], f32, name="ph", space="PSUM")
                    for k in range(KC):
                        last_mm = nc.tensor.matmul(
                            out=ph,
                            lhsT=wtv[:, k, m * P:(m + 1) * P],
                            rhs=xgv[:, k, gi * GRP:(gi + 1) * GRP],
                            start=(k == 0),
                            stop=(k == KC - 1),
                        )
                    nc.scalar.activation(
                        out=gt[:, m * GRP:(m + 1) * GRP],
                        in_=ph,
                        func=Tanh,
                        bias=bv_v[:, m:m + 1],
                        scale=1.0,
                    )
                    if filler is not None:
                        filler(m, last_mm)
                return gt, last_mm

            def emit_score(gt):
                pscore = pspool.tile([1, GRP], f32, name="pscore", space="PSUM")
                for m in range(MC):
                    nc.tensor.matmul(
                        out=pscore,
                        lhsT=qv_v[:, m:m + 1],
                        rhs=gt[:, m * GRP:(m + 1) * GRP],
                        start=(m == 0),
                        stop=(m == MC - 1),
                    )
                srow = spool.tile([1, GRP], f32, name="srow")
                nc.vector.tensor_copy(out=srow, in_=pscore)
                return srow

            def emit_transposes(srow, g):
                for j in range(JP):
                    ci = g * JP + j
                    nc.tensor.transpose(
                        out=accb[:, ci:ci + 1],
                        in_=srow[:, j * P:(j + 1) * P],
                        identity=id1_sb,
                    )

            def emit_exp(s):
                ecol = epool.tile([P, CPS], f32, name="ecol")
                nc.scalar.activation(
                    out=ecol,
                    in_=accb[:, s * CPS:(s + 1) * CPS],
                    func=Exp,
                    bias=shift_sb,
                    scale=1.0,
                )
                return ecol

            def _chunk_win(ci):
                # first/last chunk run full-width so the PSUM accumulate
                # regions get a clean start= and stop= across all columns
                if ci == 0 or ci == NCH - 1:
                    return 0, P
                w0 = _ws(ci, win)
                return w0, win

            def emit_ew_batch(ecol, s, lo, hi):
                out = []
                for i in range(lo, hi):
                    ci = s * CPS + i
                    w0, wl = _chunk_win(ci)
                    ew = ewpool.tile([P, wl], bf16, name="ew")
                    nc.vector.tensor_scalar(
                        ew,
                        iota_v[:, w0:w0 + wl],
                        segc_v[:, ci:ci + 1],
                        ecol[:, i:i + 1],
                        is_equal,
                        mult,
                    )
                    out.append(ew)
                return out

            def emit_chunk_one(ew, xnsv, s, i, after=None):
                ci = s * CPS + i
                w0, wl = _chunk_win(ci)
                first = None

                def den_mm():
                    # den rides INSIDE oacc3's open chain: start=False
                    # always; must execute after the chain opens (ci==0
                    # region mms) and before it closes (ci==NCH-1 stop)
                    return nc.tensor.matmul(
                        out=den_acc[0:1, w0:w0 + wl],
                        lhsT=ones_bf[:, 0:1],
                        rhs=ew,
                        start=False,
                        stop=False,
                    )

                if ci == NCH - 1:
                    # chain-close edge: den must precede region 3's stop
                    first = den_mm()
                mlast = None
                for m in range(MC):
                    mlast = nc.tensor.matmul(
                        out=oaccs[m][:, w0:w0 + wl],
                        lhsT=xnsv[:, i, m * P:(m + 1) * P],
                        rhs=ew,
                        start=(ci == 0),
                        stop=(ci == NCH - 1),
                    )
                    if first is None:
                        first = mlast
                if ci == NCH - 1:
                    tile.add_dep_helper(
                        mlast.ins, first.ins, sync=False,
                        reason="den rides oacc3 chain: write before close",
                    )
                else:
                    # den LAST: it then pipelines at ~25ns instead of paying
                    # the ~100ns first-after-gate turnaround (Ew is already
                    # prefetched, so nothing needs absorbing up front).
                    # At ci==0 this also orders it after the chain opens.
                    d = den_mm()
                    tile.add_dep_helper(
                        d.ins, mlast.ins, sync=False,
                        reason="den rides oacc3 chain: after open, in order",
                    )
                if after is not None:
                    # pin behind the preceding gate m-chunk in the static
                    # PE stream (see baseline kernel notes)
                    tile.add_dep_helper(
                        first.ins, after.ins, sync=False,
                        reason="chunk rides behind its gate m-chunk",
                    )

            def emit_chunk_mms(ews, xnsv, s, lo, hi, after=None):
                for i in range(lo, hi):
                    emit_chunk_one(
                        ews[i - lo], xnsv, s, i,
                        after=after if i == lo else None,
                    )

            def emit_xn_warm(xnsv):
                # PE observes the xn DMA before any chunk matmul uses xn
                # as its weights (weight-loads must never carry the only
                # unseen semaphore).
                wtx = pspool.tile([1, GRP], f32, name="pscore", space="PSUM")
                nc.tensor.matmul(
                    out=wtx[0:1, 0:2],
                    lhsT=xnsv[0:1, 0, 0:1],
                    rhs=xnsv[0:1, 0, 0:2],
                    start=True,
                    stop=True,
                )

            GPB = CPS // SUP_G  # chunks reduced per group slot (4)
            pend = None         # (xnsv, s, ecol) of the unreduced superblock
            ews_next = None     # prefetched Ew tiles for the next batch
            for s in range(NS):
                xgv = xgv0 if s == 0 else load_xt_sb(s)

                srow_prev = None
                for gi in range(SUP_G):
                    if pend is not None and gi == 0:
                        ews_next = emit_ew_batch(pend[2], pend[1], 0, GPB)
                    if pend is not None:
                        ews_cur, base, warm0 = ews_next, gi * GPB, (gi == 0)

                        def filler(mi, lastmm, ews_cur=ews_cur, base=base,
                                   warm0=warm0):
                            # one chunk's tiny flipped-wsum matmuls behind
                            # each gate m-chunk: their weight loads hide
                            # under the 213ns gate matmuls
                            if warm0 and mi == 0:
                                emit_xn_warm(pend[0])
                            emit_chunk_one(
                                ews_cur[mi], pend[0], pend[1],
                                base + mi, after=lastmm,
                            )
                    else:
                        filler = None
                    gt, last_gate = emit_gate(xgv, gi, filler)
                    if srow_prev is not None:
                        emit_transposes(*srow_prev)
                    srow = emit_score(gt)
                    if pend is not None and gi < SUP_G - 1:
                        ews_next = emit_ew_batch(
                            pend[2], pend[1],
                            (gi + 1) * GPB, (gi + 2) * GPB,
                        )
                    srow_prev = (srow, s * SUP_G + gi)
                emit_transposes(*srow_prev)
                # xn DMA issued AFTER this superblock's gates so the early
                # queue bandwidth goes to xt; consumed one superblock later
                xns = xnpool.tile([P, CPS * D], bf16, name="xns")
                nc.sync.dma_start(
                    out=xns, in_=xn[:, s * CPS * D:(s + 1) * CPS * D]
                )
                pend = (
                    xns.rearrange("p (c d) -> p c d", c=CPS), s, emit_exp(s)
                )

            emit_xn_warm(pend[0])
            ews_next = emit_ew_batch(pend[2], pend[1], 0, GPB)
            for gi in range(SUP_G):
                emit_chunk_mms(
                    ews_next, pend[0], pend[1], gi * GPB, (gi + 1) * GPB
                )
                if gi < SUP_G - 1:
                    ews_next = emit_ew_batch(
                        pend[2], pend[1], (gi + 1) * GPB, (gi + 2) * GPB
                    )

            ovt_sb = opool.tile([P, MC * P], f32, name="ovt_sb")
            for m in range(MC):
                nc.vector.tensor_copy(
                    out=ovt_sb[:, m * P:(m + 1) * P], in_=oaccs[m][:, 0:P]
                )
            od_sb = opool.tile([1, P], f32, name="od_sb")
            nc.vector.tensor_copy(out=od_sb, in_=den_acc)
            nc.sync.dma_start(out=ovt, in_=ovt_sb)
            nc.sync.dma_start(out=od, in_=od_sb)

    nc.compile()
    return nc


def _get_module(shard, win):
    key = (shard, win)
    if key not in _CACHE:
        _CACHE[key] = build_module(shard, win)
    return _CACHE[key]


def pack_consts(W, b, q, nch, bf16_dt):
    oqv, obv, osg, oio, oeye, cw = _const_layout(nch)
    cst = np.zeros((P, cw), dtype=np.float32)
    cst[:, oqv:oqv + MC] = q.reshape(MC, P).T
    cst[:, obv:obv + MC] = b.reshape(MC, P).T
    cst[:, oio:oio + P] = np.arange(P, dtype=np.float32)[None, :]
    cst[0:SUP_G, oeye:oeye + SUP_G] = np.eye(SUP_G, dtype=np.float32)
    wt = W.T.astype(np.float32)  # [k, m]
    wtb = np.ascontiguousarray(
        wt.reshape(KC, P, D).transpose(1, 0, 2).reshape(P, KC * D)
    ).astype(bf16_dt)
    return cst, wtb, osg


def pack_core(xs, seg, bf16_dt):
    """Host-side packing of one core's shard -> kernel input dict + glo."""
    shard = xs.shape[0]
    nch = shard // P
    SUP = SUP_G * GRP
    ns = shard // SUP
    cps = SUP // P
    glo = int(seg.min())
    width = int(seg.max()) - glo + 1
    assert width <= P, f"shard graph range {width} > {P} unsupported"
    rel = (seg - glo).astype(np.float32)

    xsb = xs.astype(bf16_dt)
    # xt: [128, ns, KC, SUP] -- contiguous per-superblock transposed layout
    xt = np.ascontiguousarray(
        xsb.reshape(ns, SUP, D).transpose(0, 2, 1)          # [ns, D, SUP]
        .reshape(ns, KC, P, SUP).transpose(2, 0, 1, 3)      # [P, ns, KC, SUP]
        .reshape(P, ns * KC * SUP)
    )
    # xn: [128, ns, CPS, D] -- node-major chunks, partition = node % 128
    xnp = np.ascontiguousarray(
        xsb.reshape(ns, cps, P, D).transpose(2, 0, 1, 3)    # [P, ns, cps, D]
        .reshape(P, ns * cps * D)
    )
    segc = np.ascontiguousarray(rel.reshape(nch, P).T)
    return {"xt": xt, "xn": xnp}, segc, glo, rel


def _windows_ok(rel, nch):
    """Check every chunk's graph ids fit its compile-time window."""
    r = rel.reshape(nch, P)
    lo = r.min(axis=1)
    hi = r.max(axis=1)
    for ci in range(nch):
        w0 = _ws(ci, WIN)
        if lo[ci] < w0 or hi[ci] >= w0 + WIN:
            return False
    return True


def kernel(**inputs):
    global LAST_RESULT
    import ml_dtypes
    from concourse import bass_utils

    bf16_dt = ml_dtypes.bfloat16

    x = np.ascontiguousarray(np.asarray(inputs["x"], dtype=np.float32))
    gp = np.asarray(inputs["graph_ptr"]).astype(np.int64)
    W = np.asarray(inputs["W"], dtype=np.float32)
    b = np.asarray(inputs["b"], dtype=np.float32)
    q = np.asarray(inputs["query"], dtype=np.float32)

    N = x.shape[0]
    shard = N // N_CORES
    assert N % N_CORES == 0
    nch = shard // P

    cst_base, wtb, osg = pack_consts(W, b, q, nch, bf16_dt)

    in_maps = []
    glos = []
    win_ok = True
    for c in range(N_CORES):
        per, segc, glo, rel = pack_core(
            x[c * shard:(c + 1) * shard], gp[c * shard:(c + 1) * shard], bf16_dt
        )
        win_ok = win_ok and _windows_ok(rel, nch)
        cst = cst_base.copy()
        cst[:, osg:osg + nch] = segc
        per["cst"] = cst
        per["wtb"] = wtb
        in_maps.append(per)
        glos.append(glo)

    nc = _get_module(shard, WIN if win_ok else P)
    trace = bool(int(os.environ.get("KERNEL_TRACE", "0")))
    res = bass_utils.run_bass_kernel_spmd(
        nc,
        in_maps,
        core_ids=list(range(N_CORES)),
        trace=trace,
        trace_cores=list(range(N_CORES)) if trace else None,
    )
    LAST_RESULT = res

    vec = np.zeros((G, D), dtype=np.float64)
    den = np.zeros((G,), dtype=np.float64)
    for c in range(N_CORES):
        g0 = glos[c]
        g1 = min(G, g0 + P)
        w = g1 - g0
        ovt = res.results[c]["ovt"]  # [128 d-in-chunk, MC*128 g]
        for m in range(MC):
            vec[g0:g1, m * P:(m + 1) * P] += ovt[:, m * P:m * P + w].T.astype(np.float64)
        den[g0:g1] += res.results[c]["od"][0, :w].astype(np.float64)
    den = np.where(den == 0.0, 1.0, den)
    return (vec / den[:, None]).astype(np.float32)
